# revision 53
# baseline (speedup 1.0000x reference)
"""Single-head encoder attention block on 8 Trainium2 NeuronCores.

Math (per batch element b):
    q = x @ wq.T ; k = x @ wk.T ; v = x @ wv.T
    scores = (q @ k.T) / sqrt(1024) ; attn = softmax(scores, -1)
    out = (attn @ v) @ wo.T

Sharding: data-parallel over batch - batch 8 maps 1:1 onto the 8 cores;
weights replicated. No collectives.

Weight preprocessing on host (x-independent, standard inference-time
weight folding, done once per weight set):
    M  := wq.T @ wk          so scores = x @ M @ x.T / 32
    UT := wv.T @ wo.T        so (attn @ v) @ wo.T = attn @ (x @ UT)
plus fp16 casts and the xT layout per batch element.

Per-core device algorithm (fp16 matmul operands, fp32 PSUM; all matmul
chains run at ~216 ns per 512-wide matmul; ONE [P,512]x8 PSUM pool is
shared by every phase, so no pool-close barrier - and no DVFS pstate
ramp-down after one - exists anywhere in the kernel):
  Phase F      : F[d2,i] = sum_d1 M[d1,d2] xT[d1,i].  The first i-quarter
                 runs d1-outer across 8 accumulators so the PE consumes
                 one (M[d1], xT[d1] quarter) DMA pair per 1.7us burst at
                 kernel start; quarters 1-3 run as (d2-pair, d1-inner)
                 16-matmul sub-chains.  Only the critical first DMA wave
                 is emitted before this phase - bulk xT/UT DMA triggers
                 (~0.7us each on the rings) are emitted after it.
  Phase Z      : Z[j,do] = sum_d2 xT[d2,j]^T UT[d2,do], 16-matmul chains
                 into column-pair [P,512] tiles.
  Phase scores : scoresT[j,i] = sum_d2 xT[d2,j]^T F[d2,i], 32-matmul
                 chains into four [P,512] quarter tiles (the full 8-slot
                 ring double-buffers two chains); Exp (scale 1/32) ->
                 expT fp16 resident; DVE accumulates rowsums over j.
  Phase out    : out[i,do] = (sum_j expT[j,i]^T Z[j,do]) * rcp[i], as
                 column-pair chains from the same slot ring - chain 0
                 lands in banks freed by chain j14's exp, so the PE never
                 idles at the scores->out boundary.  Rowsums: rs_acc ->
                 fp16 -> four [1,512] ones-matmuls -> DVE
                 reciprocal_approx_fast -> a [1,2048]->[128,16] relayout
                 via a DRAM-bounce DMA pair (same queue => ordered), all
                 off the PE critical path under chains 0-1.  The final
                 output scale is a Scalar Copy-activation with the
                 per-partition reciprocal; the last chain de-interleaves
                 its halves and splits the final scale/DMA into quarters
                 to shorten the tail.
"""

import os
import sys

for _p in ("/opt/trn_rl_repo", "/root/.axon_site/_ro/trn_rl_repo"):
    if os.path.isdir(_p) and _p not in sys.path:
        sys.path.insert(0, _p)

import numpy as np
from contextlib import ExitStack

import concourse.bacc as bacc
import concourse.tile as tile
from concourse import mybir
from concourse.bass_utils import run_bass_kernel_spmd

P = 128
S = 2048          # sequence length (per core)
D = 1024          # model dim = dk = dv
NS = S // P       # 16 seq tiles
ND = D // P       # 8 dim tiles
SCALE = 1.0 / 32.0  # 1/sqrt(1024)
N_CORES = 8

DT = mybir.dt.float32
MM = mybir.dt.float16
R32 = mybir.dt.float32r
F32 = mybir.dt.float32
EXP = mybir.ActivationFunctionType.Exp
COPY = mybir.ActivationFunctionType.Copy


def _build():
    nc = bacc.Bacc("TRN2", target_bir_lowering=False, debug=False, num_devices=N_CORES)

    xt_in = nc.dram_tensor("xt", [D, S], MM, kind="ExternalInput").ap()
    m_in = nc.dram_tensor("mf", [D, D], MM, kind="ExternalInput").ap()
    ut_in = nc.dram_tensor("ut", [D, D], MM, kind="ExternalInput").ap()
    out_d = nc.dram_tensor("out", [S, D], DT, kind="ExternalOutput").ap()
    rcts = nc.dram_tensor("rcts", [NS, P], DT, kind="Internal").ap()

    mm = nc.tensor.matmul

    with tile.TileContext(nc) as tc, ExitStack() as top:
        cst = top.enter_context(tc.tile_pool(name="cst", bufs=1))
        ones_f32 = cst.tile([P, 1], DT)
        ones16 = cst.tile([P, 1], MM)

        res1 = top.enter_context(tc.tile_pool(name="res1", bufs=1))
        xt = res1.tile([P, ND * S], MM)    # xT tile d -> [:, d*S:(d+1)*S] = [d-part, s]
        res2 = top.enter_context(tc.tile_pool(name="res2", bufs=1))
        zres = res2.tile([P, NS * D], MM)  # Z tile j -> [:, j*D:(j+1)*D] = [j-part, do]
        res3 = top.enter_context(tc.tile_pool(name="res3", bufs=1))
        fres = res3.tile([P, ND * S], MM)  # F tile d2 -> [:, d2*S:(d2+1)*S]
        # ONE PSUM pool for every phase: 8 x [P,512] slots, one tag. No pool
        # transition (and no close-barrier or PE pstate ramp-down) anywhere.
        ps = top.enter_context(tc.tile_pool(name="ps", bufs=8, space="PSUM"))

        # ---- input DMA rings: M[d] + xT[d] first quarter land in d order,
        # then the remaining xT quarters, then UT (first needed by phase Z).
        # gpsimd before scalar: the Scalar engine's first ~1.3us is consumed
        # by ACT_TABLE_LOAD (Exp tables), so it gets the lightest, latest-
        # needed share of the critical first DMA wave
        rings = [nc.sync.dma_start, nc.gpsimd.dma_start, nc.scalar.dma_start]

        with ExitStack() as pmu:
            wpool = pmu.enter_context(tc.tile_pool(name="wpool", bufs=1))
            mres = wpool.tile([P, ND * D], MM)   # M tile d1 -> [:, d1*D + d2]
            utres = wpool.tile([P, ND * D], MM)  # UT tile d2 -> [:, d2*D + do]

            # critical wave only: M tiles + first xT quarter, scheduled so
            # pair d1 lands just before the d1-th matmul burst of the first
            # F pass consumes it (ring transfers are ~1.4us per 128KB and
            # the three queues start staggered).  Everything else is emitted
            # after the first F pass so its trigger issue (~0.7us per DMA)
            # never gates the first matmuls.
            def dma_m(ring, d):
                ring(out=mres[:, d * D:(d + 1) * D],
                     in_=m_in[d * P:(d + 1) * P, :])

            def dma_xq0(ring, d):
                ring(out=xt[:, d * S: d * S + 512],
                     in_=xt_in[d * P:(d + 1) * P, 0:512])

            r0, r1, r2 = rings
            r0(out=mres[:, 0:512], in_=m_in[0:P, 0:512])      # M0 first half
            dma_xq0(r0, 0)
            dma_m(r0, 3); dma_xq0(r0, 3)
            dma_m(r0, 6); dma_xq0(r0, 6)
            r1(out=mres[:, 512:D], in_=m_in[0:P, 512:D])      # M0 second half
            dma_m(r1, 1); dma_xq0(r1, 1)
            dma_m(r1, 4); dma_xq0(r1, 4)
            dma_m(r1, 7); dma_xq0(r1, 7)
            dma_m(r2, 2); dma_xq0(r2, 2)
            dma_m(r2, 5); dma_xq0(r2, 5)

            with ExitStack() as pfz:
                pab = ps
                # -------- Phase F --------
                # iq=0 runs d1-outer across 8 accumulators so the PE consumes
                # one (M[d1], xT[d1] quarter) pair per 1.7us burst, matching
                # the DMA arrival rate at kernel start
                p0 = [pab.tile([P, 512], F32, name=f"f0{d2}", tag="f")
                      for d2 in range(ND)]
                for d1 in range(ND):
                    mv = xt[:, d1 * S: d1 * S + 512]
                    for d2 in range(ND):
                        mm(p0[d2][:], mres[:, d1 * D + d2 * P: d1 * D + (d2 + 1) * P],
                           mv, start=(d1 == 0), stop=(d1 == ND - 1))
                for d2 in range(ND):
                    nc.vector.tensor_copy(fres[:, d2 * S: d2 * S + 512], p0[d2][:])

                # bulk DMAs: remaining xT quarters (one DMA per tile), UT
                for d in range(ND):
                    rings[d % 3](
                        out=xt[:, d * S + 512: (d + 1) * S],
                        in_=xt_in[d * P:(d + 1) * P, 512:S])
                for d in range(ND):
                    rings[d % 3](out=utres[:, d * D:(d + 1) * D],
                                 in_=ut_in[d * P:(d + 1) * P, :])

                # constants ride behind the DMA triggers
                nc.gpsimd.memset(ones_f32[:], 1.0)
                nc.vector.tensor_copy(ones16[:], ones_f32[:])

                # iq=1..3: (d2-pair) sub-chains, d1 inner
                for iq in range(1, 4):
                    for d2h in range(4):
                        pa = pab.tile([P, 512], F32, name=f"fa{iq}{d2h}", tag="f")
                        pb = pab.tile([P, 512], F32, name=f"fb{iq}{d2h}", tag="f")
                        d2a, d2b = 2 * d2h, 2 * d2h + 1
                        for d1 in range(ND):
                            mv = xt[:, d1 * S + iq * 512: d1 * S + (iq + 1) * 512]
                            mm(pa[:], mres[:, d1 * D + d2a * P: d1 * D + (d2a + 1) * P],
                               mv, start=(d1 == 0), stop=(d1 == ND - 1))
                            mm(pb[:], mres[:, d1 * D + d2b * P: d1 * D + (d2b + 1) * P],
                               mv, start=(d1 == 0), stop=(d1 == ND - 1))
                        nc.vector.tensor_copy(
                            fres[:, d2a * S + iq * 512: d2a * S + (iq + 1) * 512], pa[:])
                        nc.vector.tensor_copy(
                            fres[:, d2b * S + iq * 512: d2b * S + (iq + 1) * 512], pb[:])

                # -------- Phase Z: j-major chains, column-pair tiles --------
                for j in range(NS):
                    za = pab.tile([P, 512], F32, name=f"za{j}", tag="f")
                    zb = pab.tile([P, 512], F32, name=f"zb{j}", tag="f")
                    for d2 in range(ND):
                        st = xt[:, d2 * S + j * P: d2 * S + (j + 1) * P]
                        mm(za[:], st, utres[:, d2 * D: d2 * D + 512],
                           start=(d2 == 0), stop=(d2 == ND - 1))
                        mm(zb[:], st, utres[:, d2 * D + 512: d2 * D + D],
                           start=(d2 == 0), stop=(d2 == ND - 1))
                    # last chain drains on the idle Scalar engine: the scores
                    # pool open waits on all Z copies, keep them off the DVE
                    if j == NS - 1:
                        nc.scalar.activation(zres[:, j * D: j * D + 512], za[:], COPY)
                        nc.scalar.activation(zres[:, j * D + 512: (j + 1) * D], zb[:], COPY)
                    else:
                        nc.vector.tensor_copy(zres[:, j * D: j * D + 512], za[:])
                        nc.vector.tensor_copy(zres[:, j * D + 512: (j + 1) * D], zb[:])

        # -------- Phase scores + out: ONE PSUM pool (tag sc, 4 x [P,1024]) ----
        # Scores chains write two [P,1024] half-tiles; out-chain psums and the
        # rowsum psums allocate from the same 4-slot ring, so the out phase
        # starts in j14's long-freed banks with no pool-close barrier (the old
        # barrier cost ~1.9us idle + ~1.7us of PE pstate ramp-down).
        with ExitStack() as pb:
            etp = pb.enter_context(tc.tile_pool(name="etp", bufs=1))
            expt = etp.tile([P, NS * S], MM)   # expT tile j -> [:, j*S + i]
            rsp = pb.enter_context(tc.tile_pool(name="rsp", bufs=1))
            rs_acc = rsp.tile([P, S], R32)
            outsb = pb.enter_context(tc.tile_pool(name="outsb", bufs=3))
            rssb = pb.enter_context(tc.tile_pool(name="rssb", bufs=2))
            rct = rssb.tile([P, NS], DT, bufs=1)  # rct[p,it]=1/rowsum[it*128+p]
            rc_sb = rssb.tile([1, S], DT, bufs=1)
            rs16 = rssb.tile([P, S], MM, bufs=1)

            for j in range(NS):
                scq = [ps.tile([P, 512], F32, name=f"sc{j}{c}", tag="f")
                       for c in range(4)]
                for d2 in range(ND):
                    st = xt[:, d2 * S + j * P: d2 * S + (j + 1) * P]
                    for c in range(4):
                        mm(scq[c][:], st,
                           fres[:, d2 * S + c * 512: d2 * S + (c + 1) * 512],
                           start=(d2 == 0), stop=(d2 == ND - 1))
                for c in range(4):
                    nc.scalar.activation(
                        expt[:, j * S + c * 512: j * S + (c + 1) * 512],
                        scq[c][:], EXP, scale=SCALE)
                if j == 0:
                    nc.vector.tensor_copy(rs_acc[:], expt[:, 0:S])
                else:
                    nc.vector.tensor_add(rs_acc[:], rs_acc[:],
                                         expt[:, j * S:(j + 1) * S])

            # ---------------- Phase out (same PSUM pool) ----------------
            def out_pair(it):
                return [ps.tile([P, 512], F32, name=f"op{it}{c}", tag="f")
                        for c in range(2)]

            def out_chain_into(op, it, interleave=True):
                # non-interleaved: half c=0 completes first so its scale and
                # DMA overlap half c=1 (used for the last chain's tail)
                cols = ([(j, c) for j in range(NS) for c in range(2)]
                        if interleave else
                        [(j, c) for c in range(2) for j in range(NS)])
                for j, c in cols:
                    mm(op[c][:],
                       expt[:, j * S + it * P: j * S + (it + 1) * P],
                       zres[:, j * D + c * 512: j * D + (c + 1) * 512],
                       start=(j == 0), stop=(j == NS - 1))

            def emit_scale(it, op):
                for c in range(2):
                    ob = outsb.tile([P, 512], DT, tag="ob")
                    nc.scalar.activation(ob[:], op[c][:], COPY,
                                         scale=rct[:, it:it + 1])
                    nc.sync.dma_start(
                        out=out_d[it * P:(it + 1) * P, c * 512:(c + 1) * 512],
                        in_=ob[:])

            # slot plan: op0/op1 pairs take j14's four slots (freed by its
            # exps long ago), the rowsum tiles take j15's (freed slice by
            # slice as its exps drain); chain2 reuses op0's slots once
            # scale0 has run - the reciprocal pipeline completes under chain1
            op0 = out_pair(0)
            op1 = out_pair(1)
            rst = [ps.tile([1, 512], F32, name=f"rs{ch}", tag="f")
                   for ch in range(4)]

            nc.vector.tensor_copy(rs16[:], rs_acc[:])
            out_chain_into(op0, 0)
            # rowsums (fp16 operands; f32r stationary fails ISA checks and
            # f32r moving pays a slow 4-byte weight load) + fast reciprocal +
            # DRAM-bounce relayout [1,2048] -> [128,16], all under chain 1
            for ch in range(4):
                mm(rst[ch][:], ones16[:, 0:1],
                   rs16[:, ch * 512:(ch + 1) * 512], start=True, stop=True)
                rs_sb = rssb.tile([1, 512], DT, name=f"rsb{ch}", tag="rs")
                nc.vector.tensor_copy(rs_sb[:], rst[ch][:])
                nc.vector.reciprocal_approx_fast(
                    out=rc_sb[:1, ch * 512:(ch + 1) * 512], in_=rs_sb[:])
            nc.gpsimd.dma_start(out=rcts[:, :], in_=rc_sb[:])
            nc.gpsimd.dma_start(out=rct[:], in_=rcts.rearrange("a b -> b a"))
            out_chain_into(op1, 1)
            emit_scale(0, op0)
            emit_scale(1, op1)
            for it in range(2, NS - 1):
                op = out_pair(it)
                out_chain_into(op, it)
                emit_scale(it, op)
            # last chain: halves de-interleaved so c0's scale+DMA overlap
            # c1's matmuls; c1's scale/DMA further split into quarters
            it = NS - 1
            op = out_pair(it)
            out_chain_into(op, it, interleave=False)
            ob = outsb.tile([P, 512], DT, tag="ob")
            nc.scalar.activation(ob[:], op[0][:], COPY, scale=rct[:, it:it + 1])
            nc.sync.dma_start(out=out_d[it * P:(it + 1) * P, 0:512], in_=ob[:])
            for q in range(2):
                obq = outsb.tile([P, 256], DT, tag="obq")
                nc.scalar.activation(obq[:], op[1][:, q * 256:(q + 1) * 256],
                                     COPY, scale=rct[:, it:it + 1])
                nc.sync.dma_start(
                    out=out_d[it * P:(it + 1) * P, 512 + q * 256: 768 + q * 256],
                    in_=obq[:])

    nc.compile()
    return nc


_NC_CACHE = None


def kernel(x, wq, wk, wv, wo):
    global _NC_CACHE
    if _NC_CACHE is None:
        _NC_CACHE = _build()
    nc = _NC_CACHE
    core_ids = list(range(N_CORES))
    # host weight folding (x-independent)
    wq32 = wq.astype(np.float32)
    wk32 = wk.astype(np.float32)
    wv32 = wv.astype(np.float32)
    wo32 = wo.astype(np.float32)
    m16 = np.ascontiguousarray((wq32.T @ wk32).astype(np.float16))
    ut16 = np.ascontiguousarray((wv32.T @ wo32.T).astype(np.float16))
    in_maps = []
    for b in range(N_CORES):
        in_maps.append({
            "xt": np.ascontiguousarray(x[b].astype(np.float16).T),
            "mf": m16,
            "ut": ut16,
        })
    res = run_bass_kernel_spmd(nc, in_maps, core_ids)
    return np.stack([res.results[b]["out"] for b in range(N_CORES)], axis=0)


# revision 54
# speedup vs baseline: 1.0075x; 1.0075x over previous
"""Single-head encoder attention block on 8 Trainium2 NeuronCores.

Math (per batch element b):
    q = x @ wq.T ; k = x @ wk.T ; v = x @ wv.T
    scores = (q @ k.T) / sqrt(1024) ; attn = softmax(scores, -1)
    out = (attn @ v) @ wo.T

Sharding: data-parallel over batch - batch 8 maps 1:1 onto the 8 cores;
weights replicated. No collectives.

Weight preprocessing on host (x-independent, standard inference-time
weight folding, done once per weight set):
    M  := wq.T @ wk          so scores = x @ M @ x.T / 32
    UT := wv.T @ wo.T        so (attn @ v) @ wo.T = attn @ (x @ UT)
plus fp16 casts and the xT layout per batch element.

Per-core device algorithm (fp16 matmul operands, fp32 PSUM; all matmul
chains run at ~216 ns per 512-wide matmul; ONE [P,512]x8 PSUM pool is
shared by every phase, so no pool-close barrier - and no DVFS pstate
ramp-down after one - exists anywhere in the kernel):
  Phase F      : F[d2,i] = sum_d1 M[d1,d2] xT[d1,i].  The first i-quarter
                 runs d1-outer across 8 accumulators so the PE consumes
                 one (M[d1], xT[d1] quarter) DMA pair per 1.7us burst at
                 kernel start; quarters 1-3 run as (d2-pair, d1-inner)
                 16-matmul sub-chains.  Only the critical first DMA wave
                 is emitted before this phase - bulk xT/UT DMA triggers
                 (~0.7us each on the rings) are emitted after it.
  Phase Z      : Z[j,do] = sum_d2 xT[d2,j]^T UT[d2,do], 16-matmul chains
                 into column-pair [P,512] tiles.
  Phase scores : scoresT[j,i] = sum_d2 xT[d2,j]^T F[d2,i], 32-matmul
                 chains into four [P,512] quarter tiles (the full 8-slot
                 ring double-buffers two chains); Exp (scale 1/32) ->
                 expT fp16 resident; DVE accumulates rowsums over j.
  Phase out    : out[i,do] = (sum_j expT[j,i]^T Z[j,do]) * rcp[i], as
                 column-pair chains from the same slot ring - chain 0
                 lands in banks freed by chain j14's exp, so the PE never
                 idles at the scores->out boundary.  Rowsums: rs_acc ->
                 fp16 -> four [1,512] ones-matmuls -> DVE
                 reciprocal_approx_fast -> a [1,2048]->[128,16] relayout
                 via a DRAM-bounce DMA pair (same queue => ordered), all
                 off the PE critical path under chains 0-1.  The final
                 output scale is a Scalar Copy-activation with the
                 per-partition reciprocal; the last chain de-interleaves
                 its halves and splits the final scale/DMA into quarters
                 to shorten the tail.
"""

import os
import sys

for _p in ("/opt/trn_rl_repo", "/root/.axon_site/_ro/trn_rl_repo"):
    if os.path.isdir(_p) and _p not in sys.path:
        sys.path.insert(0, _p)

import numpy as np
from contextlib import ExitStack

import concourse.bacc as bacc
import concourse.tile as tile
from concourse import mybir
from concourse.bass_utils import run_bass_kernel_spmd

P = 128
S = 2048          # sequence length (per core)
D = 1024          # model dim = dk = dv
NS = S // P       # 16 seq tiles
ND = D // P       # 8 dim tiles
SCALE = 1.0 / 32.0  # 1/sqrt(1024)
N_CORES = 8

DT = mybir.dt.float32
MM = mybir.dt.float16
R32 = mybir.dt.float32r
F32 = mybir.dt.float32
EXP = mybir.ActivationFunctionType.Exp
COPY = mybir.ActivationFunctionType.Copy


def _build():
    nc = bacc.Bacc("TRN2", target_bir_lowering=False, debug=False, num_devices=N_CORES)

    xt_in = nc.dram_tensor("xt", [D, S], MM, kind="ExternalInput").ap()
    m_in = nc.dram_tensor("mf", [D, D], MM, kind="ExternalInput").ap()
    ut_in = nc.dram_tensor("ut", [D, D], MM, kind="ExternalInput").ap()
    out_d = nc.dram_tensor("out", [S, D], DT, kind="ExternalOutput").ap()
    rcts = nc.dram_tensor("rcts", [NS, P], DT, kind="Internal").ap()

    mm = nc.tensor.matmul

    with tile.TileContext(nc) as tc, ExitStack() as top:
        cst = top.enter_context(tc.tile_pool(name="cst", bufs=1))
        ones_f32 = cst.tile([P, 1], DT)
        ones16 = cst.tile([P, 1], MM)

        res1 = top.enter_context(tc.tile_pool(name="res1", bufs=1))
        xt = res1.tile([P, ND * S], MM)    # xT tile d -> [:, d*S:(d+1)*S] = [d-part, s]
        res2 = top.enter_context(tc.tile_pool(name="res2", bufs=1))
        zres = res2.tile([P, NS * D], MM)  # Z tile j -> [:, j*D:(j+1)*D] = [j-part, do]
        res3 = top.enter_context(tc.tile_pool(name="res3", bufs=1))
        fres = res3.tile([P, ND * S], MM)  # F tile d2 -> [:, d2*S:(d2+1)*S]
        # ONE PSUM pool for every phase: 8 x [P,512] slots, one tag. No pool
        # transition (and no close-barrier or PE pstate ramp-down) anywhere.
        ps = top.enter_context(tc.tile_pool(name="ps", bufs=8, space="PSUM"))

        # ---- input DMA rings: M[d] + xT[d] first quarter land in d order,
        # then the remaining xT quarters, then UT (first needed by phase Z).
        rings = [nc.sync.dma_start, nc.scalar.dma_start, nc.gpsimd.dma_start]

        with ExitStack() as pmu:
            wpool = pmu.enter_context(tc.tile_pool(name="wpool", bufs=1))
            mres = wpool.tile([P, ND * D], MM)   # M tile d1 -> [:, d1*D + d2]
            utres = wpool.tile([P, ND * D], MM)  # UT tile d2 -> [:, d2*D + do]

            # critical wave only: M tiles + first xT quarter, scheduled so
            # pair d1 lands just before the d1-th matmul burst of the first
            # F pass consumes it (ring transfers are ~1.4us per 128KB and
            # the three queues start staggered).  Everything else is emitted
            # after the first F pass so its trigger issue (~0.7us per DMA)
            # never gates the first matmuls.
            def dma_m(ring, d):
                ring(out=mres[:, d * D:(d + 1) * D],
                     in_=m_in[d * P:(d + 1) * P, :])

            def dma_xq0(ring, d):
                ring(out=xt[:, d * S: d * S + 512],
                     in_=xt_in[d * P:(d + 1) * P, 0:512])

            r0, r1, r2 = rings
            r0(out=mres[:, 0:512], in_=m_in[0:P, 0:512])      # M0 first half
            dma_xq0(r0, 0)
            dma_m(r0, 3); dma_xq0(r0, 3)
            dma_m(r0, 6); dma_xq0(r0, 6)
            r1(out=mres[:, 512:D], in_=m_in[0:P, 512:D])      # M0 second half
            dma_m(r1, 1); dma_xq0(r1, 1)
            dma_m(r1, 4); dma_xq0(r1, 4)
            dma_m(r1, 7); dma_xq0(r1, 7)
            dma_m(r2, 2); dma_xq0(r2, 2)
            dma_m(r2, 5); dma_xq0(r2, 5)

            with ExitStack() as pfz:
                pab = ps
                # -------- Phase F --------
                # iq=0 runs d1-outer across 8 accumulators so the PE consumes
                # one (M[d1], xT[d1] quarter) pair per 1.7us burst, matching
                # the DMA arrival rate at kernel start
                p0 = [pab.tile([P, 512], F32, name=f"f0{d2}", tag="f")
                      for d2 in range(ND)]
                for d1 in range(ND):
                    mv = xt[:, d1 * S: d1 * S + 512]
                    for d2 in range(ND):
                        mm(p0[d2][:], mres[:, d1 * D + d2 * P: d1 * D + (d2 + 1) * P],
                           mv, start=(d1 == 0), stop=(d1 == ND - 1))
                for d2 in range(ND):
                    nc.vector.tensor_copy(fres[:, d2 * S: d2 * S + 512], p0[d2][:])

                # bulk DMAs: remaining xT quarters (one DMA per tile), UT
                for d in range(ND):
                    rings[d % 3](
                        out=xt[:, d * S + 512: (d + 1) * S],
                        in_=xt_in[d * P:(d + 1) * P, 512:S])
                for d in range(ND):
                    rings[d % 3](out=utres[:, d * D:(d + 1) * D],
                                 in_=ut_in[d * P:(d + 1) * P, :])

                # constants ride behind the DMA triggers
                nc.gpsimd.memset(ones_f32[:], 1.0)
                nc.vector.tensor_copy(ones16[:], ones_f32[:])

                # iq=1..3: (d2-pair) sub-chains, d1 inner
                for iq in range(1, 4):
                    for d2h in range(4):
                        pa = pab.tile([P, 512], F32, name=f"fa{iq}{d2h}", tag="f")
                        pb = pab.tile([P, 512], F32, name=f"fb{iq}{d2h}", tag="f")
                        d2a, d2b = 2 * d2h, 2 * d2h + 1
                        for d1 in range(ND):
                            mv = xt[:, d1 * S + iq * 512: d1 * S + (iq + 1) * 512]
                            mm(pa[:], mres[:, d1 * D + d2a * P: d1 * D + (d2a + 1) * P],
                               mv, start=(d1 == 0), stop=(d1 == ND - 1))
                            mm(pb[:], mres[:, d1 * D + d2b * P: d1 * D + (d2b + 1) * P],
                               mv, start=(d1 == 0), stop=(d1 == ND - 1))
                        nc.vector.tensor_copy(
                            fres[:, d2a * S + iq * 512: d2a * S + (iq + 1) * 512], pa[:])
                        nc.vector.tensor_copy(
                            fres[:, d2b * S + iq * 512: d2b * S + (iq + 1) * 512], pb[:])

                # -------- Phase Z: j-major chains, column-pair tiles --------
                for j in range(NS):
                    za = pab.tile([P, 512], F32, name=f"za{j}", tag="f")
                    zb = pab.tile([P, 512], F32, name=f"zb{j}", tag="f")
                    for d2 in range(ND):
                        st = xt[:, d2 * S + j * P: d2 * S + (j + 1) * P]
                        mm(za[:], st, utres[:, d2 * D: d2 * D + 512],
                           start=(d2 == 0), stop=(d2 == ND - 1))
                        mm(zb[:], st, utres[:, d2 * D + 512: d2 * D + D],
                           start=(d2 == 0), stop=(d2 == ND - 1))
                    # last chain drains on the idle Scalar engine: the scores
                    # pool open waits on all Z copies, keep them off the DVE
                    if j == NS - 1:
                        nc.scalar.activation(zres[:, j * D: j * D + 512], za[:], COPY)
                        nc.scalar.activation(zres[:, j * D + 512: (j + 1) * D], zb[:], COPY)
                    else:
                        nc.vector.tensor_copy(zres[:, j * D: j * D + 512], za[:])
                        nc.vector.tensor_copy(zres[:, j * D + 512: (j + 1) * D], zb[:])

        # -------- Phase scores + out: ONE PSUM pool (tag sc, 4 x [P,1024]) ----
        # Scores chains write two [P,1024] half-tiles; out-chain psums and the
        # rowsum psums allocate from the same 4-slot ring, so the out phase
        # starts in j14's long-freed banks with no pool-close barrier (the old
        # barrier cost ~1.9us idle + ~1.7us of PE pstate ramp-down).
        with ExitStack() as pb:
            etp = pb.enter_context(tc.tile_pool(name="etp", bufs=1))
            expt = etp.tile([P, NS * S], MM)   # expT tile j -> [:, j*S + i]
            rsp = pb.enter_context(tc.tile_pool(name="rsp", bufs=1))
            rs_acc = rsp.tile([P, S], R32)
            outsb = pb.enter_context(tc.tile_pool(name="outsb", bufs=3))
            rssb = pb.enter_context(tc.tile_pool(name="rssb", bufs=2))
            rct = rssb.tile([P, NS], DT, bufs=1)  # rct[p,it]=1/rowsum[it*128+p]
            rc_sb = rssb.tile([1, S], DT, bufs=1)
            rs16 = rssb.tile([P, S], MM, bufs=1)

            for j in range(NS):
                scq = [ps.tile([P, 512], F32, name=f"sc{j}{c}", tag="f")
                       for c in range(4)]
                for d2 in range(ND):
                    st = xt[:, d2 * S + j * P: d2 * S + (j + 1) * P]
                    for c in range(4):
                        mm(scq[c][:], st,
                           fres[:, d2 * S + c * 512: d2 * S + (c + 1) * 512],
                           start=(d2 == 0), stop=(d2 == ND - 1))
                for c in range(4):
                    nc.scalar.activation(
                        expt[:, j * S + c * 512: j * S + (c + 1) * 512],
                        scq[c][:], EXP, scale=SCALE)
                if j == 0:
                    nc.vector.tensor_copy(rs_acc[:], expt[:, 0:S])
                else:
                    nc.vector.tensor_add(rs_acc[:], rs_acc[:],
                                         expt[:, j * S:(j + 1) * S])

            # ---------------- Phase out (same PSUM pool) ----------------
            def out_pair(it):
                return [ps.tile([P, 512], F32, name=f"op{it}{c}", tag="f")
                        for c in range(2)]

            def out_chain_into(op, it, interleave=True):
                # non-interleaved: half c=0 completes first so its scale and
                # DMA overlap half c=1 (used for the last chain's tail)
                cols = ([(j, c) for j in range(NS) for c in range(2)]
                        if interleave else
                        [(j, c) for c in range(2) for j in range(NS)])
                for j, c in cols:
                    mm(op[c][:],
                       expt[:, j * S + it * P: j * S + (it + 1) * P],
                       zres[:, j * D + c * 512: j * D + (c + 1) * 512],
                       start=(j == 0), stop=(j == NS - 1))

            def emit_scale(it, op):
                for c in range(2):
                    ob = outsb.tile([P, 512], DT, tag="ob")
                    nc.scalar.activation(ob[:], op[c][:], COPY,
                                         scale=rct[:, it:it + 1])
                    nc.sync.dma_start(
                        out=out_d[it * P:(it + 1) * P, c * 512:(c + 1) * 512],
                        in_=ob[:])

            # slot plan: op0/op1 pairs take j14's four slots (freed by its
            # exps long ago), the rowsum tiles take j15's (freed slice by
            # slice as its exps drain); chain2 reuses op0's slots once
            # scale0 has run - the reciprocal pipeline completes under chain1
            op0 = out_pair(0)
            op1 = out_pair(1)
            rst = [ps.tile([1, 512], F32, name=f"rs{ch}", tag="f")
                   for ch in range(4)]

            nc.vector.tensor_copy(rs16[:], rs_acc[:])
            out_chain_into(op0, 0)
            # rowsums (fp16 operands; f32r stationary fails ISA checks and
            # f32r moving pays a slow 4-byte weight load) + fast reciprocal +
            # DRAM-bounce relayout [1,2048] -> [128,16], all under chain 1
            for ch in range(4):
                mm(rst[ch][:], ones16[:, 0:1],
                   rs16[:, ch * 512:(ch + 1) * 512], start=True, stop=True)
                rs_sb = rssb.tile([1, 512], DT, name=f"rsb{ch}", tag="rs")
                nc.vector.tensor_copy(rs_sb[:], rst[ch][:])
                nc.vector.reciprocal_approx_fast(
                    out=rc_sb[:1, ch * 512:(ch + 1) * 512], in_=rs_sb[:])
            nc.gpsimd.dma_start(out=rcts[:, :], in_=rc_sb[:])
            nc.gpsimd.dma_start(out=rct[:], in_=rcts.rearrange("a b -> b a"))
            out_chain_into(op1, 1)
            emit_scale(0, op0)
            emit_scale(1, op1)
            for it in range(2, NS - 1):
                op = out_pair(it)
                out_chain_into(op, it)
                emit_scale(it, op)
            # last chain: halves de-interleaved so c0's scale+DMA overlap
            # c1's matmuls; c1's scale/DMA further split into quarters
            it = NS - 1
            op = out_pair(it)
            out_chain_into(op, it, interleave=False)
            ob = outsb.tile([P, 512], DT, tag="ob")
            nc.scalar.activation(ob[:], op[0][:], COPY, scale=rct[:, it:it + 1])
            nc.sync.dma_start(out=out_d[it * P:(it + 1) * P, 0:512], in_=ob[:])
            for q in range(2):
                obq = outsb.tile([P, 256], DT, tag="obq")
                nc.scalar.activation(obq[:], op[1][:, q * 256:(q + 1) * 256],
                                     COPY, scale=rct[:, it:it + 1])
                nc.sync.dma_start(
                    out=out_d[it * P:(it + 1) * P, 512 + q * 256: 768 + q * 256],
                    in_=obq[:])

    nc.compile()
    return nc


_NC_CACHE = None


def kernel(x, wq, wk, wv, wo):
    global _NC_CACHE
    if _NC_CACHE is None:
        _NC_CACHE = _build()
    nc = _NC_CACHE
    core_ids = list(range(N_CORES))
    # host weight folding (x-independent)
    wq32 = wq.astype(np.float32)
    wk32 = wk.astype(np.float32)
    wv32 = wv.astype(np.float32)
    wo32 = wo.astype(np.float32)
    m16 = np.ascontiguousarray((wq32.T @ wk32).astype(np.float16))
    ut16 = np.ascontiguousarray((wv32.T @ wo32.T).astype(np.float16))
    in_maps = []
    for b in range(N_CORES):
        in_maps.append({
            "xt": np.ascontiguousarray(x[b].astype(np.float16).T),
            "mf": m16,
            "ut": ut16,
        })
    res = run_bass_kernel_spmd(nc, in_maps, core_ids)
    return np.stack([res.results[b]["out"] for b in range(N_CORES)], axis=0)
